# revision 8
# baseline (speedup 1.0000x reference)
"""Trainium2 Bass kernel for nn_DeepFCNet (similarity MLP + classification MLP).

Strategy: pure data parallel over the batch dim — each of 8 NeuronCores gets 4
subjects (x slice [4*9045, 750]) and all weights replicated; no collectives.

Per core, per 512-row tile:
  - bulk DMA x rows-major -> SBUF [128, 4, 750]
  - PE transposes (identity matmul) -> PSUM -> DVE/ACT copy -> xT [128f, 512r]
  - similarity MLP 750->32->16->8->1 on PE, feature-major, ACT fused bias+relu
  - layer 4 emits sim TRANSPOSED ([128 pairs, 4 cols]) by using h3 as the
    stationary operand, so the classification contraction needs no extra
    transpose
  - classification layer 1 (9045 -> 1024) is interleaved into the tile loop,
    accumulating in PSUM across all tiles while streaming cw1 from HBM
Tail: 1024->256->64->3 with tiny PE transposes between layers, log_softmax on
ACT/DVE, DMA out [4, 3] per core.
"""
import json as _json
import sys
from contextlib import ExitStack

sys.path.insert(0, "/opt/trn_rl_repo")

import numpy as np

import bass_rust as _bass_rust
import concourse.bass as bass
import concourse.mybir as mybir
import concourse.tile as tile
from concourse.bass import ts
from concourse.bass_utils import run_bass_kernel_spmd
from concourse.masks import make_identity

AF = mybir.ActivationFunctionType
F32 = mybir.dt.float32

# NTFF profiling glue: the image lacks antenv.axon_hooks, but the ctypes hook
# in trn_agent_boot works — shim the module so trace=True functions.
try:
    import antenv.axon_hooks  # noqa: F401
except Exception:
    try:
        import types as _types

        from trn_agent_boot.trn_boot import _ntff_profile_via_ctypes as _mk_hook

        _hook = _mk_hook("/opt/axon/libaxon_pjrt.so")
        _m = _types.ModuleType("antenv.axon_hooks")
        _m.get_axon_ntff_profile_hook = lambda: _hook
        _m.set_axon_ntff_profile_hook = lambda hook: None
        sys.modules["antenv.axon_hooks"] = _m
    except Exception:
        pass
try:
    import concourse.bass_utils as _bu

    _bu.upload_artifacts = lambda tmpdir: tmpdir
except Exception:
    pass


# ---------------------------------------------------------------------------
# Workaround: walrus on this container rejects instructions with >1 sem wait
# ("Too many sync wait commands") and the TileContext tail drain carries one
# wait per active proc.  Split it into a chain of single-wait drains.
def _split_drain_and_barrier(self, tick_clock, wait_clock):
    gc = tick_clock.global_clock
    vals = _json.loads(repr(gc).replace("VectorClock(", "").rstrip(")"))
    for i, v in enumerate(vals):
        if v > 0:
            single = [0] * len(vals)
            single[i] = v
            d = self.nc.sync.drain()
            wait_clock.add_sem_waits(
                d.ins, _bass_rust.ScopedClock({None: _bass_rust.VectorClock(single)})
            )
    self.nc.all_engine_barrier()
    assert self.sems is not None
    popped = self.nc._tile_sem_poison_stack.pop()
    assert popped is self._sem_poison
    self.nc.clear_and_free_semaphores(list(self.sems.allocated().values()))
    self.nc.all_engine_barrier()


tile.TileContext._drain_and_barrier = _split_drain_and_barrier


def _split_multi_wait_instructions(nc):
    """This container's walrus accepts at most one sem wait per instruction.
    Hoist extra waits onto engine-nops inserted immediately before the
    instruction on the same engine queue (same per-engine order, so the
    waits still complete before the instruction issues)."""
    cur_bb = nc.cur_bb.bb
    for fn in nc.m.functions:
        for bb in fn.blocks:
            il = bb.instructions
            idx = 0
            while idx < len(il):
                inst = il[idx]
                si = inst.sync_info
                if si is not None and si.on_wait and len(si.on_wait) > 1:
                    waits = list(si.on_wait)
                    ups = list(si.on_update) if si.on_update else []
                    inst.sync_info = mybir.SyncInfo(
                        on_wait=[waits[-1]], on_update=ups
                    )
                    n_added = 0
                    for w in waits[:-1]:
                        bi = nc.engines[inst.engine].nop(nofuse=True)
                        nop_inst = bi.ins
                        nop_inst.sync_info = mybir.SyncInfo(on_wait=[w], on_update=[])
                        tail = cur_bb.instructions
                        assert tail[-1] is nop_inst
                        tail.pop()
                        il.insert(idx, nop_inst)
                        n_added += 1
                    idx += n_added
                idx += 1


def _check_single_waits(nc):
    bad = []
    for fn in nc.m.functions:
        for bb in fn.blocks:
            for inst in bb.instructions:
                si = inst.sync_info
                if si is not None and si.on_wait and len(si.on_wait) > 1:
                    bad.append((inst.name, len(si.on_wait)))
    assert not bad, f"multi-wait instructions remain: {bad[:10]}"

# ---------------------------------------------------------------------------
N_CORES = 8
B = 32
P_PAIRS = 9045
F = 750
SUBJ = 4  # subjects per core
TILE_R = 512
NT = 18  # tiles per subject; 18*512 = 9216 >= 9045
NBLK = 4 * NT  # 72 row-blocks of 128 per (padded) subject
NJ_FULL = 70  # 9045 = 70*128 + 85
K_LAST = 85
ROWS_PAD = (SUBJ - 1) * P_PAIRS + NT * TILE_R  # 36351 padded rows per core
FC = [128, 128, 128, 128, 128, 110]  # feature chunks of 750
DVE_COPY_CHUNKS = (0, 1, 2, 3)  # xT psum->sbuf copies on DVE; rest on ACT


def _bcast(dram_handle, p):
    """AP reading a 1-D DRAM tensor broadcast across p partitions."""
    ap = dram_handle[:]
    return bass.AP(tensor=ap.tensor, offset=ap.offset, ap=[[0, p]] + list(ap.ap))


def build_nc():
    nc = bass.Bass()
    xd = nc.declare_dram_parameter("x", [ROWS_PAD, F], F32, isOutput=False)
    sw1 = nc.declare_dram_parameter("sw1", [F, 32], F32, isOutput=False)
    sb1 = nc.declare_dram_parameter("sb1", [32], F32, isOutput=False)
    sw2 = nc.declare_dram_parameter("sw2", [32, 16], F32, isOutput=False)
    sb2 = nc.declare_dram_parameter("sb2", [16], F32, isOutput=False)
    sw3 = nc.declare_dram_parameter("sw3", [16, 8], F32, isOutput=False)
    sb3 = nc.declare_dram_parameter("sb3", [8], F32, isOutput=False)
    sw4 = nc.declare_dram_parameter("sw4", [8, 1], F32, isOutput=False)
    sb4 = nc.declare_dram_parameter("sb4", [1], F32, isOutput=False)
    cw1 = nc.declare_dram_parameter("cw1", [P_PAIRS, 1024], F32, isOutput=False)
    cb1 = nc.declare_dram_parameter("cb1", [1024], F32, isOutput=False)
    cw2 = nc.declare_dram_parameter("cw2", [1024, 256], F32, isOutput=False)
    cb2 = nc.declare_dram_parameter("cb2", [256], F32, isOutput=False)
    cw3 = nc.declare_dram_parameter("cw3", [256, 64], F32, isOutput=False)
    cb3 = nc.declare_dram_parameter("cb3", [64], F32, isOutput=False)
    cw4 = nc.declare_dram_parameter("cw4", [64, 3], F32, isOutput=False)
    cb4 = nc.declare_dram_parameter("cb4", [3], F32, isOutput=False)
    outd = nc.declare_dram_parameter("out", [SUBJ, 3], F32, isOutput=True)

    with tile.TileContext(nc) as tc, ExitStack() as ctx:
        consts = ctx.enter_context(tc.tile_pool(name="consts", bufs=1))
        xin = ctx.enter_context(tc.tile_pool(name="xin", bufs=2))
        xtp = ctx.enter_context(tc.tile_pool(name="xtp", bufs=2))
        hp = ctx.enter_context(tc.tile_pool(name="hp", bufs=2))
        simp = ctx.enter_context(tc.tile_pool(name="simp", bufs=1))
        cw1p = ctx.enter_context(tc.tile_pool(name="cw1p", bufs=4))
        clsp = ctx.enter_context(tc.tile_pool(name="clsp", bufs=1))
        ps_xt = ctx.enter_context(tc.tile_pool(name="ps_xt", bufs=2, space="PSUM"))
        ps_h1 = ctx.enter_context(tc.tile_pool(name="ps_h1", bufs=1, space="PSUM"))
        ps_h2 = ctx.enter_context(tc.tile_pool(name="ps_h2", bufs=1, space="PSUM"))
        ps_h3 = ctx.enter_context(tc.tile_pool(name="ps_h3", bufs=1, space="PSUM"))
        ps_sim = ctx.enter_context(tc.tile_pool(name="ps_sim", bufs=1, space="PSUM"))
        ps_c1 = ctx.enter_context(tc.tile_pool(name="ps_c1", bufs=1, space="PSUM"))

        # ---- constants ----
        ident = consts.tile([128, 128], F32)
        make_identity(nc, ident)
        w1s = consts.tile([128, 6, 32], F32)
        for c, kc in enumerate(FC):
            nc.gpsimd.dma_start(w1s[:kc, c, :], sw1[c * 128 : c * 128 + kc, :])
        w2s = consts.tile([32, 16], F32)
        nc.gpsimd.dma_start(w2s[:], sw2[:, :])
        w3s = consts.tile([16, 8], F32)
        nc.gpsimd.dma_start(w3s[:], sw3[:, :])
        w4s = consts.tile([8, 1], F32)
        nc.gpsimd.dma_start(w4s[:], sw4[:, :])
        b1s = consts.tile([32, 1], F32)
        nc.gpsimd.dma_start(b1s[:], sb1[:].rearrange("(p o) -> p o", o=1))
        b2s = consts.tile([16, 1], F32)
        nc.gpsimd.dma_start(b2s[:], sb2[:].rearrange("(p o) -> p o", o=1))
        b3s = consts.tile([8, 1], F32)
        nc.gpsimd.dma_start(b3s[:], sb3[:].rearrange("(p o) -> p o", o=1))
        b4s = consts.tile([128, 1], F32)
        nc.gpsimd.dma_start(b4s[:], _bcast(sb4, 128))
        cw2s = consts.tile([128, 8, 256], F32)
        nc.sync.dma_start(cw2s[:], cw2[:, :].rearrange("(k p) n -> p k n", p=128))
        cw3s = consts.tile([128, 2, 64], F32)
        nc.sync.dma_start(cw3s[:], cw3[:, :].rearrange("(k p) n -> p k n", p=128))
        cw4s = consts.tile([64, 3], F32)
        nc.gpsimd.dma_start(cw4s[:], cw4[:, :])
        cb1s = consts.tile([4, 1024], F32)
        nc.gpsimd.dma_start(cb1s[:], _bcast(cb1, 4))
        cb2s = consts.tile([4, 256], F32)
        nc.gpsimd.dma_start(cb2s[:], _bcast(cb2, 4))
        cb3s = consts.tile([4, 64], F32)
        nc.gpsimd.dma_start(cb3s[:], _bcast(cb3, 4))
        cb4s = consts.tile([4, 3], F32)
        nc.gpsimd.dma_start(cb4s[:], _bcast(cb4, 4))

        simT = simp.tile([128, SUBJ, NBLK], F32)
        c1a = ps_c1.tile([4, 512], F32, tag="c1a")
        c1b = ps_c1.tile([4, 512], F32, tag="c1b")

        # ---- main loop ----
        for t in range(NT):
            for s in range(SUBJ):
                r0 = s * P_PAIRS + t * TILE_R
                xtile = xin.tile([128, 4, F], F32, tag="xtile")
                nc.sync.dma_start(
                    xtile[:],
                    xd[r0 : r0 + TILE_R, :].rearrange("(b p) f -> p b f", p=128),
                )
                xt = xtp.tile([128, 6, TILE_R], F32, tag="xt")
                for c, kc in enumerate(FC):
                    pxt = ps_xt.tile([128, TILE_R], F32, tag="pxt")
                    for b in range(4):
                        # regular matmul x_chunk.T @ I (not transpose-mode):
                        # numerically identical, but counts as PE-busy so the
                        # HAM clock gate stays at full rate
                        nc.tensor.matmul(
                            pxt[:kc, ts(b, 128)],
                            xtile[:, b, c * 128 : c * 128 + kc],
                            ident,
                            start=True, stop=True,
                        )
                    if c in DVE_COPY_CHUNKS:
                        nc.vector.tensor_copy(xt[:kc, c, :], pxt[:kc, :])
                    else:
                        nc.scalar.activation(xt[:kc, c, :], pxt[:kc, :], AF.Copy)

                ph1 = ps_h1.tile([32, TILE_R], F32, tag="ph1")
                for c, kc in enumerate(FC):
                    nc.tensor.matmul(
                        ph1[:], w1s[:kc, c, :], xt[:kc, c, :],
                        start=(c == 0), stop=(c == 5),
                    )
                h1 = hp.tile([32, TILE_R], F32, tag="h1")
                nc.scalar.activation(h1[:], ph1[:], AF.Relu, bias=b1s[:])

                ph2 = ps_h2.tile([16, TILE_R], F32, tag="ph2")
                nc.tensor.matmul(ph2[:], w2s[:], h1[:], start=True, stop=True)
                h2 = hp.tile([16, TILE_R], F32, tag="h2")
                nc.scalar.activation(h2[:], ph2[:], AF.Relu, bias=b2s[:])

                ph3 = ps_h3.tile([8, TILE_R], F32, tag="ph3")
                nc.tensor.matmul(ph3[:], w3s[:], h2[:], start=True, stop=True)
                h3 = hp.tile([8, TILE_R], F32, tag="h3")
                nc.scalar.activation(h3[:], ph3[:], AF.Relu, bias=b3s[:])

                psim = ps_sim.tile([128, 4], F32, tag="psim")
                for b in range(4):
                    nc.tensor.matmul(
                        psim[:, b : b + 1], h3[:, ts(b, 128)], w4s[:],
                        start=True, stop=True,
                    )
                nc.scalar.activation(
                    simT[:, s, 4 * t : 4 * t + 4], psim[:], AF.Tanh, bias=b4s[:]
                )

            # classification layer 1, interleaved: contraction chunks for this t
            for jj in range(4):
                j = 4 * t + jj
                if j > NJ_FULL:
                    continue
                kj = 128 if j < NJ_FULL else K_LAST
                cwt = cw1p.tile([128, 1024], F32, tag="cwt")
                nc.sync.dma_start(cwt[:kj, :], cw1[j * 128 : j * 128 + kj, :])
                nc.tensor.matmul(
                    c1a[:], simT[:kj, :, j], cwt[:kj, 0:512],
                    start=(j == 0), stop=(j == NJ_FULL),
                )
                nc.tensor.matmul(
                    c1b[:], simT[:kj, :, j], cwt[:kj, 512:1024],
                    start=(j == 0), stop=(j == NJ_FULL),
                )

        # ---- classification tail ----
        c1 = clsp.tile([4, 1024], F32)
        nc.vector.tensor_add(c1[:, 0:512], c1a[:], cb1s[:, 0:512])
        nc.vector.tensor_add(c1[:, 512:1024], c1b[:], cb1s[:, 512:1024])
        nc.vector.tensor_scalar_max(c1[:], c1[:], 0.0)

        c1T = clsp.tile([128, 8, 4], F32)
        for k in range(8):
            pxt = ps_xt.tile([128, TILE_R], F32, tag="pxt")
            nc.tensor.transpose(pxt[:, 0:4], c1[:, ts(k, 128)], ident[0:4, 0:4])
            nc.vector.tensor_copy(c1T[:, k, :], pxt[:, 0:4])

        pc2 = ps_h1.tile([32, TILE_R], F32, tag="ph1")
        for k in range(8):
            nc.tensor.matmul(
                pc2[0:4, 0:256], c1T[:, k, :], cw2s[:, k, :],
                start=(k == 0), stop=(k == 7),
            )
        c2 = clsp.tile([4, 256], F32)
        nc.vector.tensor_add(c2[:], pc2[0:4, 0:256], cb2s[:])
        nc.vector.tensor_scalar_max(c2[:], c2[:], 0.0)

        c2T = clsp.tile([128, 2, 4], F32)
        for k in range(2):
            pxt = ps_xt.tile([128, TILE_R], F32, tag="pxt")
            nc.tensor.transpose(pxt[:, 0:4], c2[:, ts(k, 128)], ident[0:4, 0:4])
            nc.vector.tensor_copy(c2T[:, k, :], pxt[:, 0:4])

        pc3 = ps_h2.tile([16, TILE_R], F32, tag="ph2")
        for k in range(2):
            nc.tensor.matmul(
                pc3[0:4, 0:64], c2T[:, k, :], cw3s[:, k, :],
                start=(k == 0), stop=(k == 1),
            )
        c3 = clsp.tile([4, 64], F32)
        nc.vector.tensor_add(c3[:], pc3[0:4, 0:64], cb3s[:])
        nc.vector.tensor_scalar_max(c3[:], c3[:], 0.0)

        c3T = clsp.tile([64, 4], F32)
        pxt = ps_xt.tile([128, TILE_R], F32, tag="pxt")
        nc.tensor.transpose(pxt[:64, 0:4], c3[:, 0:64], ident[0:4, 0:4])
        nc.vector.tensor_copy(c3T[:], pxt[:64, 0:4])

        pc4 = ps_h3.tile([8, TILE_R], F32, tag="ph3")
        nc.tensor.matmul(pc4[0:4, 0:3], c3T[:], cw4s[:], start=True, stop=True)
        logits = clsp.tile([4, 3], F32)
        nc.vector.tensor_add(logits[:], pc4[0:4, 0:3], cb4s[:])

        # log_softmax along the free dim (3)
        m = clsp.tile([4, 1], F32)
        nc.vector.reduce_max(m[:], logits[:], axis=mybir.AxisListType.X)
        negm = clsp.tile([4, 1], F32)
        nc.scalar.mul(negm[:], m[:], -1.0)
        exps = clsp.tile([4, 3], F32)
        sume = clsp.tile([4, 1], F32)
        nc.scalar.activation(exps[:], logits[:], AF.Exp, bias=negm[:], accum_out=sume[:])
        lse = clsp.tile([4, 1], F32)
        nc.scalar.activation(lse[:], sume[:], AF.Ln)
        tot = clsp.tile([4, 1], F32)
        nc.vector.tensor_add(tot[:], m[:], lse[:])
        osb = clsp.tile([4, 3], F32)
        nc.vector.tensor_scalar_sub(osb[:], logits[:], tot[:])
        nc.sync.dma_start(outd[:, :], osb[:])

    _split_multi_wait_instructions(nc)
    _check_single_waits(nc)
    return nc


_NC = None
LAST_EXEC_NS = None
TRACE = False


def kernel(x, sw1, sb1, sw2, sb2, sw3, sb3, sw4, sb4,
           cw1, cb1, cw2, cb2, cw3, cb3, cw4, cb4):
    global _NC, LAST_EXEC_NS
    if _NC is None:
        _NC = build_nc()

    x = np.ascontiguousarray(np.asarray(x, dtype=np.float32))
    x_flat = x.reshape(B * P_PAIRS, F)
    weights = dict(
        sw1=np.asarray(sw1, np.float32), sb1=np.asarray(sb1, np.float32),
        sw2=np.asarray(sw2, np.float32), sb2=np.asarray(sb2, np.float32),
        sw3=np.asarray(sw3, np.float32), sb3=np.asarray(sb3, np.float32),
        sw4=np.asarray(sw4, np.float32), sb4=np.asarray(sb4, np.float32),
        cw1=np.asarray(cw1, np.float32), cb1=np.asarray(cb1, np.float32),
        cw2=np.asarray(cw2, np.float32), cb2=np.asarray(cb2, np.float32),
        cw3=np.asarray(cw3, np.float32), cb3=np.asarray(cb3, np.float32),
        cw4=np.asarray(cw4, np.float32), cb4=np.asarray(cb4, np.float32),
    )
    rows_per_core = SUBJ * P_PAIRS
    in_maps = []
    for c in range(N_CORES):
        lo = c * rows_per_core
        hi = min(lo + ROWS_PAD, B * P_PAIRS)
        xc = np.zeros((ROWS_PAD, F), dtype=np.float32)
        xc[: hi - lo] = x_flat[lo:hi]
        in_maps.append({"x": xc, **weights})

    tmpdir = None
    if TRACE:
        import tempfile

        tmpdir = tempfile.mkdtemp(prefix="ktrace_")
        print(f"trace dir: {tmpdir}")
    res = run_bass_kernel_spmd(
        _NC, in_maps, list(range(N_CORES)), trace=TRACE, tmpdir=tmpdir
    )
    LAST_EXEC_NS = res.exec_time_ns
    out = np.concatenate([res.results[c]["out"] for c in range(N_CORES)], axis=0)
    return out.astype(np.float32)


# revision 11
# speedup vs baseline: 1.9357x; 1.9357x over previous
"""Trainium2 Bass kernel for nn_DeepFCNet (similarity MLP + classification MLP).

Strategy: pure data parallel over the batch dim — each of 8 NeuronCores gets 4
subjects (x slice [4*9045, 750]) and all weights replicated; no collectives.

Per core, per 512-row tile:
  - bulk DMA x rows-major -> SBUF [128, 4, 750]
  - PE transposes (identity matmul) -> PSUM -> DVE/ACT copy -> xT [128f, 512r]
  - similarity MLP 750->32->16->8->1 on PE, feature-major, ACT fused bias+relu
  - layer 4 emits sim TRANSPOSED ([128 pairs, 4 cols]) by using h3 as the
    stationary operand, so the classification contraction needs no extra
    transpose
  - classification layer 1 (9045 -> 1024) is interleaved into the tile loop,
    accumulating in PSUM across all tiles while streaming cw1 from HBM
Tail: 1024->256->64->3 with tiny PE transposes between layers, log_softmax on
ACT/DVE, DMA out [4, 3] per core.
"""
import json as _json
import sys
from contextlib import ExitStack

sys.path.insert(0, "/opt/trn_rl_repo")

import numpy as np

import bass_rust as _bass_rust
import concourse.bass as bass
import concourse.mybir as mybir
import concourse.tile as tile
from concourse.bass import ts
from concourse.bass_utils import run_bass_kernel_spmd
from concourse.masks import make_identity

AF = mybir.ActivationFunctionType
F32 = mybir.dt.float32
BF16 = mybir.dt.float16  # 2-byte PE fast path; fp16 mantissa beats bf16 8x

# NTFF profiling glue: the image lacks antenv.axon_hooks, but the ctypes hook
# in trn_agent_boot works — shim the module so trace=True functions.
try:
    import antenv.axon_hooks  # noqa: F401
except Exception:
    try:
        import types as _types

        from trn_agent_boot.trn_boot import _ntff_profile_via_ctypes as _mk_hook

        _hook = _mk_hook("/opt/axon/libaxon_pjrt.so")
        _m = _types.ModuleType("antenv.axon_hooks")
        _m.get_axon_ntff_profile_hook = lambda: _hook
        _m.set_axon_ntff_profile_hook = lambda hook: None
        sys.modules["antenv.axon_hooks"] = _m
    except Exception:
        pass
try:
    import concourse.bass_utils as _bu

    _bu.upload_artifacts = lambda tmpdir: tmpdir
except Exception:
    pass


# ---------------------------------------------------------------------------
# Workaround: walrus on this container rejects instructions with >1 sem wait
# ("Too many sync wait commands") and the TileContext tail drain carries one
# wait per active proc.  Split it into a chain of single-wait drains.
def _split_drain_and_barrier(self, tick_clock, wait_clock):
    gc = tick_clock.global_clock
    vals = _json.loads(repr(gc).replace("VectorClock(", "").rstrip(")"))
    for i, v in enumerate(vals):
        if v > 0:
            single = [0] * len(vals)
            single[i] = v
            d = self.nc.sync.drain()
            wait_clock.add_sem_waits(
                d.ins, _bass_rust.ScopedClock({None: _bass_rust.VectorClock(single)})
            )
    self.nc.all_engine_barrier()
    assert self.sems is not None
    popped = self.nc._tile_sem_poison_stack.pop()
    assert popped is self._sem_poison
    self.nc.clear_and_free_semaphores(list(self.sems.allocated().values()))
    self.nc.all_engine_barrier()


tile.TileContext._drain_and_barrier = _split_drain_and_barrier


def _split_multi_wait_instructions(nc):
    """This container's walrus accepts at most one sem wait per instruction.
    Hoist extra waits onto engine-nops inserted immediately before the
    instruction on the same engine queue (same per-engine order, so the
    waits still complete before the instruction issues)."""
    cur_bb = nc.cur_bb.bb
    for fn in nc.m.functions:
        for bb in fn.blocks:
            il = bb.instructions
            idx = 0
            while idx < len(il):
                inst = il[idx]
                si = inst.sync_info
                if si is not None and si.on_wait and len(si.on_wait) > 1:
                    waits = list(si.on_wait)
                    ups = list(si.on_update) if si.on_update else []
                    inst.sync_info = mybir.SyncInfo(
                        on_wait=[waits[-1]], on_update=ups
                    )
                    n_added = 0
                    for w in waits[:-1]:
                        bi = nc.engines[inst.engine].nop(nofuse=True)
                        nop_inst = bi.ins
                        nop_inst.sync_info = mybir.SyncInfo(on_wait=[w], on_update=[])
                        tail = cur_bb.instructions
                        assert tail[-1] is nop_inst
                        tail.pop()
                        il.insert(idx, nop_inst)
                        n_added += 1
                    idx += n_added
                idx += 1


def _check_single_waits(nc):
    bad = []
    for fn in nc.m.functions:
        for bb in fn.blocks:
            for inst in bb.instructions:
                si = inst.sync_info
                if si is not None and si.on_wait and len(si.on_wait) > 1:
                    bad.append((inst.name, len(si.on_wait)))
    assert not bad, f"multi-wait instructions remain: {bad[:10]}"

# ---------------------------------------------------------------------------
N_CORES = 8
B = 32
P_PAIRS = 9045
F = 750
SUBJ = 4  # subjects per core
TILE_R = 512
NT = 18  # tiles per subject; 18*512 = 9216 >= 9045
NBLK = 4 * NT  # 72 row-blocks of 128 per (padded) subject
NJ_FULL = 70  # 9045 = 70*128 + 85
K_LAST = 85
ROWS_PAD = (SUBJ - 1) * P_PAIRS + NT * TILE_R  # 36351 padded rows per core
FC = [128, 128, 128, 128, 128, 110]  # feature chunks of 750
DVE_COPY_CHUNKS = (0, 1, 2, 3)  # xT psum->sbuf copies on DVE; rest on ACT


def _bcast(dram_handle, p):
    """AP reading a 1-D DRAM tensor broadcast across p partitions."""
    ap = dram_handle[:]
    return bass.AP(tensor=ap.tensor, offset=ap.offset, ap=[[0, p]] + list(ap.ap))


def build_nc():
    nc = bass.Bass()
    xd = nc.declare_dram_parameter("x", [ROWS_PAD, F], F32, isOutput=False)
    sw1 = nc.declare_dram_parameter("sw1", [F, 32], F32, isOutput=False)
    sb1 = nc.declare_dram_parameter("sb1", [32], F32, isOutput=False)
    sw2 = nc.declare_dram_parameter("sw2", [32, 16], F32, isOutput=False)
    sb2 = nc.declare_dram_parameter("sb2", [16], F32, isOutput=False)
    sw3 = nc.declare_dram_parameter("sw3", [16, 8], F32, isOutput=False)
    sb3 = nc.declare_dram_parameter("sb3", [8], F32, isOutput=False)
    sw4 = nc.declare_dram_parameter("sw4", [8, 1], F32, isOutput=False)
    sb4 = nc.declare_dram_parameter("sb4", [1], F32, isOutput=False)
    cw1 = nc.declare_dram_parameter("cw1", [P_PAIRS, 1024], F32, isOutput=False)
    cb1 = nc.declare_dram_parameter("cb1", [1024], F32, isOutput=False)
    cw2 = nc.declare_dram_parameter("cw2", [1024, 256], F32, isOutput=False)
    cb2 = nc.declare_dram_parameter("cb2", [256], F32, isOutput=False)
    cw3 = nc.declare_dram_parameter("cw3", [256, 64], F32, isOutput=False)
    cb3 = nc.declare_dram_parameter("cb3", [64], F32, isOutput=False)
    cw4 = nc.declare_dram_parameter("cw4", [64, 3], F32, isOutput=False)
    cb4 = nc.declare_dram_parameter("cb4", [3], F32, isOutput=False)
    outd = nc.declare_dram_parameter("out", [SUBJ, 3], F32, isOutput=True)

    with tile.TileContext(nc) as tc, ExitStack() as ctx:
        consts = ctx.enter_context(tc.tile_pool(name="consts", bufs=1))
        xin = ctx.enter_context(tc.tile_pool(name="xin", bufs=2))
        xtp = ctx.enter_context(tc.tile_pool(name="xtp", bufs=2))
        hp = ctx.enter_context(tc.tile_pool(name="hp", bufs=2))
        simp = ctx.enter_context(tc.tile_pool(name="simp", bufs=1))
        cw1p = ctx.enter_context(tc.tile_pool(name="cw1p", bufs=4))
        clsp = ctx.enter_context(tc.tile_pool(name="clsp", bufs=1))
        ps_xt = ctx.enter_context(tc.tile_pool(name="ps_xt", bufs=2, space="PSUM"))
        ps_h1 = ctx.enter_context(tc.tile_pool(name="ps_h1", bufs=1, space="PSUM"))
        ps_h2 = ctx.enter_context(tc.tile_pool(name="ps_h2", bufs=1, space="PSUM"))
        ps_h3 = ctx.enter_context(tc.tile_pool(name="ps_h3", bufs=1, space="PSUM"))
        ps_sim = ctx.enter_context(tc.tile_pool(name="ps_sim", bufs=1, space="PSUM"))
        ps_c1 = ctx.enter_context(tc.tile_pool(name="ps_c1", bufs=1, space="PSUM"))

        # ---- constants ----
        ident = consts.tile([128, 128], BF16)
        make_identity(nc, ident)
        identf = consts.tile([8, 8], F32)
        make_identity(nc, identf)
        w1s = consts.tile([128, 6, 32], BF16)
        for c, kc in enumerate(FC):
            nc.gpsimd.dma_start(w1s[:kc, c, :], sw1[c * 128 : c * 128 + kc, :])
        w2s = consts.tile([32, 16], BF16)
        nc.gpsimd.dma_start(w2s[:], sw2[:, :])
        w3s = consts.tile([16, 8], BF16)
        nc.gpsimd.dma_start(w3s[:], sw3[:, :])
        w4s = consts.tile([8, 1], BF16)
        nc.gpsimd.dma_start(w4s[:], sw4[:, :])
        b1s = consts.tile([32, 1], F32)
        nc.gpsimd.dma_start(b1s[:], sb1[:].rearrange("(p o) -> p o", o=1))
        b2s = consts.tile([16, 1], F32)
        nc.gpsimd.dma_start(b2s[:], sb2[:].rearrange("(p o) -> p o", o=1))
        b3s = consts.tile([8, 1], F32)
        nc.gpsimd.dma_start(b3s[:], sb3[:].rearrange("(p o) -> p o", o=1))
        b4s = consts.tile([128, 1], F32)
        nc.gpsimd.dma_start(b4s[:], _bcast(sb4, 128))
        cw2s = consts.tile([128, 8, 256], F32)
        nc.sync.dma_start(cw2s[:], cw2[:, :].rearrange("(k p) n -> p k n", p=128))
        cw3s = consts.tile([128, 2, 64], F32)
        nc.sync.dma_start(cw3s[:], cw3[:, :].rearrange("(k p) n -> p k n", p=128))
        cw4s = consts.tile([64, 3], F32)
        nc.gpsimd.dma_start(cw4s[:], cw4[:, :])
        cb1s = consts.tile([4, 1024], F32)
        nc.gpsimd.dma_start(cb1s[:], _bcast(cb1, 4))
        cb2s = consts.tile([4, 256], F32)
        nc.gpsimd.dma_start(cb2s[:], _bcast(cb2, 4))
        cb3s = consts.tile([4, 64], F32)
        nc.gpsimd.dma_start(cb3s[:], _bcast(cb3, 4))
        cb4s = consts.tile([4, 3], F32)
        nc.gpsimd.dma_start(cb4s[:], _bcast(cb4, 4))

        simT = simp.tile([128, SUBJ, NBLK], BF16)
        c1a = ps_c1.tile([4, 512], F32, tag="c1a")
        c1b = ps_c1.tile([4, 512], F32, tag="c1b")

        # ---- main loop ----
        for t in range(NT):
            for s in range(SUBJ):
                r0 = s * P_PAIRS + t * TILE_R
                xtile = xin.tile([128, 4, F], BF16, tag="xtile")
                nc.gpsimd.dma_start(
                    xtile[:],
                    xd[r0 : r0 + TILE_R, :].rearrange("(b p) f -> p b f", p=128),
                )
                xt = xtp.tile([128, 6, TILE_R], BF16, tag="xt")
                for c, kc in enumerate(FC):
                    pxt = ps_xt.tile([128, TILE_R], BF16, tag="pxt")
                    for b in range(4):
                        nc.tensor.transpose(
                            pxt[:kc, ts(b, 128)],
                            xtile[:, b, c * 128 : c * 128 + kc],
                            ident,
                        )
                    if c in DVE_COPY_CHUNKS:
                        nc.vector.tensor_copy(xt[:kc, c, :], pxt[:kc, :])
                    else:
                        nc.scalar.activation(xt[:kc, c, :], pxt[:kc, :], AF.Copy)

                ph1 = ps_h1.tile([32, TILE_R], F32, tag="ph1")
                for c, kc in enumerate(FC):
                    nc.tensor.matmul(
                        ph1[:], w1s[:kc, c, :], xt[:kc, c, :],
                        start=(c == 0), stop=(c == 5),
                    )
                h1 = hp.tile([32, TILE_R], BF16, tag="h1")
                nc.scalar.activation(h1[:], ph1[:], AF.Relu, bias=b1s[:])

                ph2 = ps_h2.tile([16, TILE_R], F32, tag="ph2")
                nc.tensor.matmul(ph2[:], w2s[:], h1[:], start=True, stop=True)
                h2 = hp.tile([16, TILE_R], BF16, tag="h2")
                nc.scalar.activation(h2[:], ph2[:], AF.Relu, bias=b2s[:])

                ph3 = ps_h3.tile([8, TILE_R], F32, tag="ph3")
                nc.tensor.matmul(ph3[:], w3s[:], h2[:], start=True, stop=True)
                h3 = hp.tile([8, TILE_R], BF16, tag="h3")
                nc.scalar.activation(h3[:], ph3[:], AF.Relu, bias=b3s[:])

                psim = ps_sim.tile([128, 4], F32, tag="psim")
                for b in range(4):
                    nc.tensor.matmul(
                        psim[:, b : b + 1], h3[:, ts(b, 128)], w4s[:],
                        start=True, stop=True,
                    )
                nc.scalar.activation(
                    simT[:, s, 4 * t : 4 * t + 4], psim[:], AF.Tanh, bias=b4s[:]
                )

            # classification layer 1, interleaved: contraction chunks for this t
            for jj in range(4):
                j = 4 * t + jj
                if j > NJ_FULL:
                    continue
                kj = 128 if j < NJ_FULL else K_LAST
                cwt = cw1p.tile([128, 1024], BF16, tag="cwt")
                nc.gpsimd.dma_start(cwt[:kj, :], cw1[j * 128 : j * 128 + kj, :])
                nc.tensor.matmul(
                    c1a[:], simT[:kj, :, j], cwt[:kj, 0:512],
                    start=(j == 0), stop=(j == NJ_FULL),
                )
                nc.tensor.matmul(
                    c1b[:], simT[:kj, :, j], cwt[:kj, 512:1024],
                    start=(j == 0), stop=(j == NJ_FULL),
                )

        # ---- classification tail ----
        c1 = clsp.tile([4, 1024], F32)
        nc.vector.tensor_add(c1[:, 0:512], c1a[:], cb1s[:, 0:512])
        nc.vector.tensor_add(c1[:, 512:1024], c1b[:], cb1s[:, 512:1024])
        nc.vector.tensor_scalar_max(c1[:], c1[:], 0.0)

        c1T = clsp.tile([128, 8, 4], F32)
        for k in range(8):
            pxts = ps_sim.tile([128, 4], F32, tag="psim")
            nc.tensor.transpose(pxts[:], c1[:, ts(k, 128)], identf[0:4, 0:4])
            nc.vector.tensor_copy(c1T[:, k, :], pxts[:])

        pc2 = ps_h1.tile([32, TILE_R], F32, tag="ph1")
        for k in range(8):
            nc.tensor.matmul(
                pc2[0:4, 0:256], c1T[:, k, :], cw2s[:, k, :],
                start=(k == 0), stop=(k == 7),
            )
        c2 = clsp.tile([4, 256], F32)
        nc.vector.tensor_add(c2[:], pc2[0:4, 0:256], cb2s[:])
        nc.vector.tensor_scalar_max(c2[:], c2[:], 0.0)

        c2T = clsp.tile([128, 2, 4], F32)
        for k in range(2):
            pxts = ps_sim.tile([128, 4], F32, tag="psim")
            nc.tensor.transpose(pxts[:], c2[:, ts(k, 128)], identf[0:4, 0:4])
            nc.vector.tensor_copy(c2T[:, k, :], pxts[:])

        pc3 = ps_h2.tile([16, TILE_R], F32, tag="ph2")
        for k in range(2):
            nc.tensor.matmul(
                pc3[0:4, 0:64], c2T[:, k, :], cw3s[:, k, :],
                start=(k == 0), stop=(k == 1),
            )
        c3 = clsp.tile([4, 64], F32)
        nc.vector.tensor_add(c3[:], pc3[0:4, 0:64], cb3s[:])
        nc.vector.tensor_scalar_max(c3[:], c3[:], 0.0)

        c3T = clsp.tile([64, 4], F32)
        pxts = ps_sim.tile([128, 4], F32, tag="psim")
        nc.tensor.transpose(pxts[:64, :], c3[:, 0:64], identf[0:4, 0:4])
        nc.vector.tensor_copy(c3T[:], pxts[:64, :])

        pc4 = ps_h3.tile([8, TILE_R], F32, tag="ph3")
        nc.tensor.matmul(pc4[0:4, 0:3], c3T[:], cw4s[:], start=True, stop=True)
        logits = clsp.tile([4, 3], F32)
        nc.vector.tensor_add(logits[:], pc4[0:4, 0:3], cb4s[:])

        # log_softmax along the free dim (3)
        m = clsp.tile([4, 1], F32)
        nc.vector.reduce_max(m[:], logits[:], axis=mybir.AxisListType.X)
        negm = clsp.tile([4, 1], F32)
        nc.scalar.mul(negm[:], m[:], -1.0)
        exps = clsp.tile([4, 3], F32)
        sume = clsp.tile([4, 1], F32)
        nc.scalar.activation(exps[:], logits[:], AF.Exp, bias=negm[:], accum_out=sume[:])
        lse = clsp.tile([4, 1], F32)
        nc.scalar.activation(lse[:], sume[:], AF.Ln)
        tot = clsp.tile([4, 1], F32)
        nc.vector.tensor_add(tot[:], m[:], lse[:])
        osb = clsp.tile([4, 3], F32)
        nc.vector.tensor_scalar_sub(osb[:], logits[:], tot[:])
        nc.sync.dma_start(outd[:, :], osb[:])

    _split_multi_wait_instructions(nc)
    _check_single_waits(nc)
    return nc


_NC = None
LAST_EXEC_NS = None
TRACE = False


def kernel(x, sw1, sb1, sw2, sb2, sw3, sb3, sw4, sb4,
           cw1, cb1, cw2, cb2, cw3, cb3, cw4, cb4):
    global _NC, LAST_EXEC_NS
    if _NC is None:
        _NC = build_nc()

    x = np.ascontiguousarray(np.asarray(x, dtype=np.float32))
    x_flat = x.reshape(B * P_PAIRS, F)
    weights = dict(
        sw1=np.asarray(sw1, np.float32), sb1=np.asarray(sb1, np.float32),
        sw2=np.asarray(sw2, np.float32), sb2=np.asarray(sb2, np.float32),
        sw3=np.asarray(sw3, np.float32), sb3=np.asarray(sb3, np.float32),
        sw4=np.asarray(sw4, np.float32), sb4=np.asarray(sb4, np.float32),
        cw1=np.asarray(cw1, np.float32), cb1=np.asarray(cb1, np.float32),
        cw2=np.asarray(cw2, np.float32), cb2=np.asarray(cb2, np.float32),
        cw3=np.asarray(cw3, np.float32), cb3=np.asarray(cb3, np.float32),
        cw4=np.asarray(cw4, np.float32), cb4=np.asarray(cb4, np.float32),
    )
    rows_per_core = SUBJ * P_PAIRS
    in_maps = []
    for c in range(N_CORES):
        lo = c * rows_per_core
        hi = min(lo + ROWS_PAD, B * P_PAIRS)
        xc = np.zeros((ROWS_PAD, F), dtype=np.float32)
        xc[: hi - lo] = x_flat[lo:hi]
        in_maps.append({"x": xc, **weights})

    tmpdir = None
    if TRACE:
        import tempfile

        tmpdir = tempfile.mkdtemp(prefix="ktrace_")
        print(f"trace dir: {tmpdir}")
    res = run_bass_kernel_spmd(
        _NC, in_maps, list(range(N_CORES)), trace=TRACE, tmpdir=tmpdir
    )
    LAST_EXEC_NS = res.exec_time_ns
    out = np.concatenate([res.results[c]["out"] for c in range(N_CORES)], axis=0)
    return out.astype(np.float32)
